# revision 42
# baseline (speedup 1.0000x reference)
"""Multi-head attention (QKV proj + SDPA + output proj) on 8 Trainium2 cores.

Sharding: tensor-parallel over heads. 16 heads / 8 cores = 2 heads per core.
Each core computes q/k/v for its 2 heads, SDPA, and a partial output
projection against its 128-column slice of proj_w. The host sums the 8
partial projections (the all-reduce step done host-side, since this kernel
returns full outputs anyway).

The kernel is ACT-bound: softmax needs 2 heads x 4096^2 = 33.5M exp()
elements per core through the scalar engine at 128 lanes * 1.2 GHz
(~260us floor), so the whole schedule exists to keep the ACT exp stream
gapless from ~6us onward:

  - scores land in PSUM in alternating chunks A=[128,2048] (4 banks) /
    B=[128,1024] (2 banks) holding the (j,h)-unit stream j-major h-minor,
    so one chunk's exp overlaps the next chunk's score matmuls. Score
    matmuls for the two heads are adjacent -> they pair on PE row-quadrants.
  - AV matmuls are col-tiled: h0 -> output partitions 0..63, h1 -> 64..127
    (tile_position=(0, h*64)), adjacent pairs run concurrently on the PE
    column halves and both heads share ONE av PSUM bank.
  - softmax denominators: per-(i,h) running sums of the e tiles on the
    (otherwise idle) vector engine in bf16, then one ones-matmul per (i,h)
    reduces across partitions (exact, fp32 PSUM). This removes the
    65-column "ones trick" that kept AV at half array utilization.
  - k/v/q projections and the output projection are interleaved into the
    attention phase as slot-gated "side work"; AV pops lag the exp stream
    elastically (big lag early) so i0's PE slack absorbs the one-time
    k/v/q work without stalling ACT.
  - PSUM: scA 4 + scB 2 + av/den 1 + util 1 = 8 banks.

Softmax skips the max-subtraction: scores have std ~1 (scale=1/8, d=64,
unit-variance q/k), so exp() stays in fp32 range with huge margin.

The v bias and proj bias are linear post-terms: attn weights sum to 1, so
v_bias contributes qkv_b[2048:] @ proj_w.T to every row — added on host.
"""

from collections import deque

import numpy as np
import ml_dtypes

N_CORES = 8
SEQ = 4096
DMODEL = 1024
NHEADS = 16
DHEAD = 64
H_PER_CORE = NHEADS // N_CORES  # 2
CBLK = DMODEL // N_CORES  # 128 head-dim columns per core

IT = 512  # i (query) tile width
NI = SEQ // IT  # 8
JT = 128  # j (key) tile = psum partition dim
NJ = SEQ // JT  # 32
NCT = DMODEL // 128  # 8 contraction tiles for the projections
JBLK = 512  # DMA/k-proj j-block width
NJB = SEQ // JBLK  # 8
SCALE = DHEAD ** -0.5

CSZ_A, CSZ_B = 4, 2  # alternating sc chunk sizes, in (j,h) units

DEBUG_TAPS = False  # extra DRAM outputs for stage-by-stage debugging

_CACHE = {}


def _chunk_pattern():
    """Split the 64 (j,h)-units of one i-tile into alternating A/B chunks."""
    units = [(j, h) for j in range(NJ) for h in range(2)]
    chunks = []
    pos = 0
    use_a = True
    while pos < len(units):
        sz = CSZ_A if use_a else CSZ_B
        sz = min(sz, len(units) - pos)
        chunks.append(("A" if use_a else "B", units[pos : pos + sz]))
        pos += sz
        use_a = not use_a
    return chunks


def _build_nc():
    import concourse.tile as tile
    from concourse import bacc, mybir

    bf16 = mybir.dt.bfloat16
    f16 = mybir.dt.float16
    f32 = mybir.dt.float32
    Exp = mybir.ActivationFunctionType.Exp

    nc = bacc.Bacc(
        "TRN2",
        target_bir_lowering=False,
        debug=False,
        enable_asserts=True,
        num_devices=N_CORES,
    )

    # partition-major host layouts so each load is ONE big DMA (DMA issue
    # on a queue costs ~650ns per instruction; many small DMAs starve the
    # prologue):
    #   xTr[p, jb*4096 + c*512 + col] = x.T[c*128+p, jb*512+col]
    #   wqkr[p, c*256 + u]            = wqk[c*128+p, u]
    #   wvr[p, c*128 + u]             = wv[c*128+p, u]
    xTr = nc.dram_tensor("xTr", [128, NCT * SEQ], bf16, kind="ExternalInput").ap()
    wqkr = nc.dram_tensor("wqkr", [128, NCT * 256], bf16, kind="ExternalInput").ap()
    wvr = nc.dram_tensor("wvr", [128, NCT * CBLK], bf16, kind="ExternalInput").ap()
    pw = nc.dram_tensor("pw", [CBLK, DMODEL], bf16, kind="ExternalInput").ap()
    bqk = nc.dram_tensor("bqk", [128, 2], f32, kind="ExternalInput").ap()
    partialT = nc.dram_tensor(
        "partialT", [DMODEL, SEQ], f16, kind="ExternalOutput"
    ).ap()
    if DEBUG_TAPS:
        dbg_kT = nc.dram_tensor("dbg_kT", [128, SEQ], bf16, kind="ExternalOutput").ap()
        dbg_e = nc.dram_tensor("dbg_e", [128, 4 * IT], bf16, kind="ExternalOutput").ap()
        dbg_av = nc.dram_tensor("dbg_av", [128, IT], f32, kind="ExternalOutput").ap()
        dbg_den = nc.dram_tensor("dbg_den", [16, IT], f32, kind="ExternalOutput").ap()
        dbg_rb = nc.dram_tensor("dbg_rb", [16, IT], f32, kind="ExternalOutput").ap()
        dbg_dacc = nc.dram_tensor("dbg_dacc", [16, IT], bf16, kind="ExternalOutput").ap()
        dbg_ao = nc.dram_tensor("dbg_ao", [128, SEQ], bf16, kind="ExternalOutput").ap()

    chunks = _chunk_pattern()

    with tile.TileContext(nc) as tc:
        with (
            tc.tile_pool(name="weights", bufs=1) as wpool,
            tc.tile_pool(name="xtiles", bufs=1) as xpool,
            tc.tile_pool(name="kq", bufs=1) as kqpool,
            tc.tile_pool(name="qtiles", bufs=3) as qpool,
            tc.tile_pool(name="vt", bufs=NJ) as vpool,
            tc.tile_pool(name="expA", bufs=10) as eapool,
            tc.tile_pool(name="expB", bufs=9) as ebpool,
            tc.tile_pool(name="dacc", bufs=8) as dpool,
            tc.tile_pool(name="attn", bufs=1) as apool,
            tc.tile_pool(name="norm", bufs=2) as npool,
            tc.tile_pool(name="stage", bufs=6) as stpool,
            tc.tile_pool(name="psA", bufs=1, space="PSUM") as psa,
            tc.tile_pool(name="psB", bufs=1, space="PSUM") as psb,
            tc.tile_pool(name="psV", bufs=1, space="PSUM") as psv,
            tc.tile_pool(name="psU", bufs=1, space="PSUM") as psu,
        ):
            # ---- ACT table warm-up: trigger the exp table load at t~0 ----
            warm_in = stpool.tile([1, 16], f32, name="warm_in")
            nc.vector.memset(warm_in[:], 0.0)
            warm_out = stpool.tile([1, 16], f32, name="warm_out")
            nc.scalar.activation(warm_out[:], warm_in[:], Exp)
            ones_t = wpool.tile([128, 1], bf16, name="ones_t")
            nc.vector.memset(ones_t[:], 1.0)

            # ---- weight + x DMAs: few big transfers, spread over 4 queues
            XB = NCT * JBLK  # 4096 columns of xt_all per j-block
            wqk_all = wpool.tile([128, NCT * 256], bf16, name="wqk_all")
            nc.sync.dma_start(wqk_all[:], wqkr[:])
            bqk_t = wpool.tile([128, 2], f32)
            nc.gpsimd.dma_start(bqk_t[:], bqk[:])  # tiny; gpsimd swdge is slow
            xt_all = xpool.tile([128, NCT * SEQ], bf16, name="xt_all", tag="xc")
            # jb0 in halves on the two fast queues: k(0)/q(0) start earlier
            nc.sync.dma_start(xt_all[:, 0 : XB // 2], xTr[:, 0 : XB // 2])
            nc.scalar.dma_start(xt_all[:, XB // 2 : XB], xTr[:, XB // 2 : XB])
            nc.scalar.dma_start(xt_all[:, XB : 2 * XB], xTr[:, XB : 2 * XB])
            wv_all = wpool.tile([128, NCT * CBLK], bf16, name="wv_all")
            nc.sync.dma_start(wv_all[:], wvr[:])
            pw_t = wpool.tile([128, DMODEL], bf16)
            nc.scalar.dma_start(pw_t[:], pw[:])
            dma_qs = [nc.sync, nc.scalar]
            for jb in range(2, NJB):
                dma_qs[jb % 2].dma_start(
                    xt_all[:, jb * XB : (jb + 1) * XB], xTr[:, jb * XB : (jb + 1) * XB]
                )

            def x_sl(c, jb, off, width):
                base = jb * XB + c * JBLK + off
                return xt_all[:, base : base + width]

            kT = kqpool.tile([128, SEQ], bf16)
            attn_outT = apool.tile([128, SEQ], bf16)
            qT = {}  # i -> tile
            vt = {}  # j -> tile [128, 128] = [v_h0 | v_h1]

            # rotating 1-bank psum tiles ("uv" doubles as the av/den bank)
            def util_tile(tag, name):
                pool = {"uv": psv, "u2": psu}[tag]
                return pool.tile([128, IT], f32, name=name, tag=tag)

            def _kq_mms(ps, woff, jb, lo, hi):
                for c in range(lo, hi):
                    nc.tensor.matmul(
                        ps[:],
                        wqk_all[:, c * 256 + woff : c * 256 + woff + 128],
                        x_sl(c, jb, 0, JBLK),
                        start=(c == 0),
                        stop=(c == NCT - 1),
                    )

            def emit_k(jb, tag):
                ps = util_tile(tag, f"kps{jb}")
                _kq_mms(ps, 128, jb, 0, NCT)
                nc.vector.tensor_scalar_add(
                    kT[:, jb * JBLK : (jb + 1) * JBLK], ps[:], bqk_t[:, 1:2]
                )

            def emit_q(i, tag):
                ps = util_tile(tag, f"qps{i}")
                _kq_mms(ps, 0, i, 0, NCT)
                qt = qpool.tile([128, IT], bf16, name=f"qT{i}", tag="qT")
                nc.vector.tensor_scalar_add(qt[:], ps[:], bqk_t[:, 0:1])
                qT[i] = qt

            def split_kq(kind, key, tag):
                """(first, second) side-work closures: 4 c-steps each, so a
                single pop never inserts a >1us matmul burst into the PE
                FIFO. First allocates the psum tile; second drains."""
                box = {}
                woff = 128 if kind == "k" else 0

                def first():
                    box["ps"] = util_tile(tag, f"{kind}ps{key}")
                    _kq_mms(box["ps"], woff, key, 0, 4)

                def second():
                    ps = box["ps"]
                    _kq_mms(ps, woff, key, 4, NCT)
                    if kind == "k":
                        nc.vector.tensor_scalar_add(
                            kT[:, key * JBLK : (key + 1) * JBLK], ps[:], bqk_t[:, 1:2]
                        )
                    else:
                        qt = qpool.tile([128, IT], bf16, name=f"qT{key}", tag="qT")
                        nc.vector.tensor_scalar_add(qt[:], ps[:], bqk_t[:, 0:1])
                        qT[key] = qt

                return first, second

            def emit_v(j, tag):
                ps = util_tile(tag, f"vps{j}")
                for c in range(NCT):
                    nc.tensor.matmul(
                        ps[:, 0:CBLK],
                        x_sl(c, j // 4, (j % 4) * JT, JT),
                        wv_all[:, c * CBLK : (c + 1) * CBLK],
                        start=(c == 0),
                        stop=(c == NCT - 1),
                    )
                va = vpool.tile([128, CBLK], bf16, name=f"vt{j}", tag="vt")
                nc.vector.tensor_copy(va[:], ps[:, 0:CBLK])
                vt[j] = va

            def emit_proj(cc, i, tag, use_act=False):
                ps = util_tile(tag, f"pp{cc}_{i}")
                nc.tensor.matmul(
                    ps[:],
                    pw_t[:, cc * 128 : (cc + 1) * 128],
                    attn_outT[:, i * IT : (i + 1) * IT],
                    start=True,
                    stop=True,
                )
                st = stpool.tile([128, IT], f16, name=f"st{cc}_{i}", tag="st")
                if use_act:  # tail only: ACT is idle once the exps are done
                    nc.scalar.copy(st[:], ps[:])
                else:
                    nc.vector.tensor_copy(st[:], ps[:])
                nc.sync.dma_start(
                    partialT[cc * 128 : (cc + 1) * 128, i * IT : (i + 1) * IT],
                    st[:],
                )

            # ---- prologue: k(0), q(0), v(0..3) only — everything else is
            # side work, so the PE FIFO reaches i0's score matmuls early.
            emit_k(0, "u2")
            emit_q(0, "uv")
            for j in range(4):
                emit_v(j, ["u2", "uv"][j % 2])

            # side work during the attention phase: runs on util bank u2
            # ONLY ("uv" is the av/den bank). Entries are (ready_slot,
            # closure): a closure is not popped before the global chunk-slot
            # counter reaches ready_slot, so work gated on slow chains
            # (normalize) never parks in the PE FIFO ahead of ready
            # attention matmuls.
            side_work = deque()
            slot = [0]
            # interleave k(1..7) with v(4..17): scores need k(jb) by slot
            # ~2.6*jb, av (elastically lagged) needs v(j) much later.
            for step in range(7):
                a, b = split_kq("k", step + 1, "u2")
                side_work.append((0, a))
                side_work.append((0, b))
                for j in (4 + 2 * step, 5 + 2 * step):
                    side_work.append((0, lambda j=j: emit_v(j, "u2")))
            a, b = split_kq("q", 1, "u2")
            side_work.append((0, a))
            side_work.append((0, b))
            for j in range(18, NJ):
                side_work.append((max(0, j - 14), lambda j=j: emit_v(j, "u2")))

            # ---- attention main loop ----
            av = {}  # i -> psum tile (h0 rows 0:64, h1 rows 64:128)
            den_acc = {}  # (i, h) -> sbuf bf16 running sum over j
            pending_av = deque()  # (i, e_tile, units)

            def emit_av_chunk(item):
                i, e_t, units = item
                for t, (j, h) in enumerate(units):
                    if i not in av:
                        av[i] = psv.tile([128, IT], f32, name=f"av_{i}", tag="uv")
                    # per-head accumulation groups: the has_written clear on
                    # start=True is per written partition-range, so each
                    # head must start its own group (measured: a shared
                    # group leaves h1 accumulating onto the previous
                    # i-tile's values).
                    nc.tensor.matmul(
                        av[i][h * 64 : (h + 1) * 64, :],
                        vt[j][:, h * 64 : h * 64 + 64],
                        e_t[:, t * IT : (t + 1) * IT],
                        start=(j == 0),
                        stop=(j == NJ - 1),
                        tile_position=(0, h * 64),
                    )
                    if j == NJ - 1 and h == 1:
                        finish_i(i)

            def finish_i(i):
                last = i == NI - 1
                # one copy drains both heads' av (frees the uv bank for
                # av(i+1)); the whole normalize chain is slot-delayed so no
                # engine FIFO ever head-blocks on the den accumulators
                # (which trail in the DVE/Pool queues).
                avs = npool.tile([128, IT], f32, name=f"avs_{i}", tag="avs")
                if last:
                    nc.scalar.copy(avs[:], av[i][:])
                else:
                    nc.vector.tensor_copy(avs[:], av[i][:])
                del av[i]
                if DEBUG_TAPS and i == 0:
                    nc.sync.dma_start(dbg_av[:], avs[:])
                rb = npool.tile([128, IT], f32, name=f"rb_{i}", tag="rb")

                def den_chain(i=i, rb=rb):
                    denp = psu.tile([128, IT], f32, name=f"den_{i}", tag="u2")
                    for h in range(2):
                        nc.tensor.matmul(
                            denp[h * 32 : h * 32 + 1, :],
                            ones_t[:, 0:1],
                            den_acc[(i, h, "v")][:],
                            start=True,
                            stop=False,
                            tile_position=(0, h * 32),
                        )
                        nc.tensor.matmul(
                            denp[h * 32 : h * 32 + 1, :],
                            ones_t[:, 0:1],
                            den_acc[(i, h, "p")][:],
                            start=False,
                            stop=True,
                            tile_position=(0, h * 32),
                        )
                    for h in range(2):
                        # den to partition 0 SBUF: reciprocal_approx_fast
                        # (custom DVE op) misreads partition-offset inputs.
                        dent = npool.tile([1, IT], f32, name=f"den{h}_{i}", tag="dent")
                        if last and h == 1:
                            nc.scalar.copy(dent[:], denp[h * 32 : h * 32 + 1, :])
                        else:
                            nc.vector.tensor_copy(dent[:], denp[h * 32 : h * 32 + 1, :])
                        rd = npool.tile([1, IT], f32, name=f"rd{h}_{i}", tag="rd")
                        nc.vector.reciprocal_approx_fast(rd[:], dent[:])
                        if h == 0:
                            nc.gpsimd.partition_broadcast(
                                rb[0:64, :], rd[:, 0:IT], channels=64
                            )
                        else:
                            # partition_broadcast mis-writes base-64 outputs;
                            # broadcast at base 0 and copy up.
                            rbt = npool.tile([64, IT], f32, name=f"rbt_{i}", tag="rbt")
                            nc.gpsimd.partition_broadcast(
                                rbt[:], rd[:, 0:IT], channels=64
                            )
                            nc.vector.tensor_copy(rb[64:128, :], rbt[:])
                        if DEBUG_TAPS:
                            nc.sync.dma_start(
                                dbg_den[2 * i + h : 2 * i + h + 1, :], dent[:]
                            )

                def mul_norm(i=i, avs=avs, rb=rb):
                    nc.vector.tensor_mul(
                        attn_outT[:, i * IT : (i + 1) * IT], avs[:], rb[:]
                    )

                if last:
                    den_chain()
                    mul_norm()
                else:
                    side_work.append((slot[0] + 3, den_chain))
                    if i + 2 < NI:
                        a, b = split_kq("q", i + 2, "u2")
                        side_work.append((slot[0] + 4, a))
                        side_work.append((slot[0] + 5, b))
                    side_work.append((slot[0] + 6, mul_norm))
                rdy = slot[0] + 8
                if last:
                    # tail: uv bank is free after the den matmuls — rotate
                    # proj across both util banks and alternate the drain
                    # between DVE and the now-idle ACT engine.
                    rot = ["u2", "uv"]
                    for cc in range(NCT):
                        side_work.append(
                            (
                                slot[0],
                                lambda cc=cc, i=i: emit_proj(
                                    cc, i, rot[cc % 2], use_act=(cc % 2 == 1)
                                ),
                            )
                        )
                else:
                    # stagger: >=2 chunk-slots between proj matmuls so they
                    # never crowd the PE FIFO ahead of the next scores
                    for cc in range(NCT):
                        side_work.append(
                            (rdy + 2 * cc, lambda cc=cc, i=i: emit_proj(cc, i, "u2"))

                        )

            for i in range(NI):
                # elastic av lag: defer av matmuls early so i0/i1 PE slack
                # absorbs the one-time k/v/q side work without stalling ACT
                lag = {0: 8, 1: 6, 2: 4}.get(i, 3)
                for ck, (kind, units) in enumerate(chunks):
                    csz = len(units)
                    if kind == "A":
                        sc = psa.tile(
                            [128, CSZ_A * IT], f32, name=f"scA_{i}_{ck}", tag="scA"
                        )
                    else:
                        sc = psb.tile(
                            [128, CSZ_B * IT], f32, name=f"scB_{i}_{ck}", tag="scB"
                        )
                    for t, (j, h) in enumerate(units):
                        nc.tensor.matmul(
                            sc[:, t * IT : (t + 1) * IT],
                            kT[h * 64 : (h + 1) * 64, j * JT : (j + 1) * JT],
                            qT[i][h * 64 : (h + 1) * 64, :],
                            start=True,
                            stop=True,
                            tile_position=(h * 64, 0),
                        )
                    epool = eapool if kind == "A" else ebpool
                    e_t = epool.tile(
                        [128, csz * IT], bf16, name=f"e_{i}_{ck}", tag=f"e{kind}"
                    )
                    nc.scalar.activation(
                        e_t[:, 0 : csz * IT], sc[:, 0 : csz * IT], Exp, scale=SCALE
                    )
                    if DEBUG_TAPS and i == 0 and ck == 0:
                        nc.sync.dma_start(dbg_e[:], e_t[:])
                    # denominator running sums, split DVE (j%3!=2) and the
                    # otherwise-idle gpsimd engine (j%3==2) into two partial
                    # accumulators; the fp32 ones-matmul pair sums them and
                    # reduces across partitions exactly.
                    for t, (j, h) in enumerate(units):
                        sl = e_t[:, t * IT : (t + 1) * IT]
                        eng, key = (
                            (nc.gpsimd, "p") if j % 10 in (3, 6, 9) else (nc.vector, "v")
                        )
                        if (i, h, key) not in den_acc:
                            da = dpool.tile(
                                [128, IT], bf16, name=f"dacc{key}{h}_{i}", tag="dacc"
                            )
                            eng.tensor_copy(da[:], sl)
                            den_acc[(i, h, key)] = da
                        else:
                            da = den_acc[(i, h, key)]
                            eng.tensor_add(da[:], da[:], sl)
                    pending_av.append((i, e_t, units))
                    navpop = 0
                    while len(pending_av) > lag and navpop < 2:
                        emit_av_chunk(pending_av.popleft())
                        navpop += 1
                    slot[0] += 1
                    npop = 2 if i == 0 else 1
                    for _ in range(npop):
                        if side_work and side_work[0][0] <= slot[0]:
                            side_work.popleft()[1]()

            while pending_av:
                emit_av_chunk(pending_av.popleft())
                slot[0] += 1
                if side_work and side_work[0][0] <= slot[0]:
                    side_work.popleft()[1]()
            while side_work:
                side_work.popleft()[1]()
            if DEBUG_TAPS:
                nc.sync.dma_start(dbg_kT[:], kT[:])
                nc.sync.dma_start(dbg_ao[:], attn_outT[:])

    nc.compile()
    return nc


def _get_nc():
    if "nc" not in _CACHE:
        _CACHE["nc"] = _build_nc()
    return _CACHE["nc"]


def build_in_maps(x, qkv_w, qkv_b, proj_w):
    """Per-core input dicts in the device's partition-major layouts:
    xTr[p, jb*4096+c*512+col] = x.T[c*128+p, jb*512+col], and the c-chunks
    of wqk/wv concatenated along columns with partitions inside each chunk.
    """
    bf16 = ml_dtypes.bfloat16
    x2d = np.ascontiguousarray(np.asarray(x).reshape(SEQ, DMODEL).T)  # [1024, 4096]
    xTr = np.ascontiguousarray(
        x2d.reshape(NCT, 128, NJB, JBLK).transpose(1, 2, 0, 3).reshape(128, NCT * SEQ)
    ).astype(bf16)

    def cmajor(w):  # [1024, u] -> [128, 8*u]
        u = w.shape[1]
        return np.ascontiguousarray(
            np.asarray(w).reshape(NCT, 128, u).transpose(1, 0, 2).reshape(128, NCT * u)
        ).astype(bf16)

    in_maps = []
    for c in range(N_CORES):
        lo, hi = c * CBLK, (c + 1) * CBLK
        wq_c = np.asarray(qkv_w[lo:hi, :])  # [128, 1024]
        wk_c = np.asarray(qkv_w[DMODEL + lo : DMODEL + hi, :])
        wv_c = np.asarray(qkv_w[2 * DMODEL + lo : 2 * DMODEL + hi, :])
        in_maps.append(
            {
                "xTr": xTr,
                "wqkr": cmajor(np.concatenate([wq_c.T, wk_c.T], axis=1)),
                "wvr": cmajor(wv_c.T),
                "pw": np.ascontiguousarray(np.asarray(proj_w)[:, lo:hi].T).astype(
                    bf16
                ),
                "bqk": np.ascontiguousarray(
                    np.stack(
                        [qkv_b[lo:hi], qkv_b[DMODEL + lo : DMODEL + hi]], axis=1
                    )
                ).astype(np.float32),
            }
        )
    return in_maps


def kernel(x, qkv_w, qkv_b, proj_w, proj_b):
    from concourse.bass_utils import run_bass_kernel_spmd

    nc = _get_nc()
    in_maps = build_in_maps(x, qkv_w, qkv_b, proj_w)
    res = run_bass_kernel_spmd(nc, in_maps, core_ids=list(range(N_CORES)))

    acc = np.zeros((DMODEL, SEQ), dtype=np.float32)
    for c in range(N_CORES):
        acc += res.results[c]["partialT"].astype(np.float32)

    # host-side linear bias terms: proj bias + v-bias routed through proj
    bias = qkv_b[2 * DMODEL :].astype(np.float32) @ proj_w.T.astype(
        np.float32
    ) + proj_b.astype(np.float32)
    out = acc.T + bias[None, :]
    return out.reshape(1, SEQ, DMODEL).astype(np.float32)


# revision 43
# speedup vs baseline: 1.3822x; 1.3822x over previous
"""Multi-head attention (QKV proj + SDPA + output proj) on 8 Trainium2 cores.

Sharding: tensor-parallel over heads. 16 heads / 8 cores = 2 heads per core.
Each core computes q/k/v for its 2 heads, SDPA, and a partial output
projection against its 128-column slice of proj_w. The host sums the 8
partial projections (the all-reduce step done host-side, since this kernel
returns full outputs anyway).

The kernel is ACT-bound: softmax needs 2 heads x 4096^2 = 33.5M exp()
elements per core through the scalar engine at 128 lanes * 1.2 GHz
(~260us floor), so the whole schedule exists to keep the ACT exp stream
gapless from ~6us onward:

  - scores land in PSUM in alternating chunks A=[128,2048] (4 banks) /
    B=[128,1024] (2 banks) holding the (j,h)-unit stream j-major h-minor,
    so one chunk's exp overlaps the next chunk's score matmuls. Score
    matmuls for the two heads are adjacent -> they pair on PE row-quadrants.
  - AV matmuls are col-tiled: h0 -> output partitions 0..63, h1 -> 64..127
    (tile_position=(0, h*64)), adjacent pairs run concurrently on the PE
    column halves and both heads share ONE av PSUM bank.
  - softmax denominators: per-(i,h) running sums of the e tiles on the
    (otherwise idle) vector engine in bf16, then one ones-matmul per (i,h)
    reduces across partitions (exact, fp32 PSUM). This removes the
    65-column "ones trick" that kept AV at half array utilization.
  - k/v/q projections and the output projection are interleaved into the
    attention phase as slot-gated "side work"; AV pops lag the exp stream
    elastically (big lag early) so i0's PE slack absorbs the one-time
    k/v/q work without stalling ACT.
  - PSUM: scA 4 + scB 2 + av/den 1 + util 1 = 8 banks.

Softmax skips the max-subtraction: scores have std ~1 (scale=1/8, d=64,
unit-variance q/k), so exp() stays in fp32 range with huge margin.

The v bias and proj bias are linear post-terms: attn weights sum to 1, so
v_bias contributes qkv_b[2048:] @ proj_w.T to every row — added on host.
"""

from collections import deque

import numpy as np
import ml_dtypes

N_CORES = 8
SEQ = 4096
DMODEL = 1024
NHEADS = 16
DHEAD = 64
H_PER_CORE = NHEADS // N_CORES  # 2
CBLK = DMODEL // N_CORES  # 128 head-dim columns per core

IT = 512  # i (query) tile width
NI = SEQ // IT  # 8
JT = 128  # j (key) tile = psum partition dim
NJ = SEQ // JT  # 32
NCT = DMODEL // 128  # 8 contraction tiles for the projections
JBLK = 512  # DMA/k-proj j-block width
NJB = SEQ // JBLK  # 8
SCALE = DHEAD ** -0.5

CSZ_A, CSZ_B = 4, 2  # alternating sc chunk sizes, in (j,h) units

DEBUG_TAPS = False  # extra DRAM outputs for stage-by-stage debugging

_CACHE = {}


def _chunk_pattern():
    """Split the 64 (j,h)-units of one i-tile into alternating A/B chunks."""
    units = [(j, h) for j in range(NJ) for h in range(2)]
    chunks = []
    pos = 0
    use_a = True
    while pos < len(units):
        sz = CSZ_A if use_a else CSZ_B
        sz = min(sz, len(units) - pos)
        chunks.append(("A" if use_a else "B", units[pos : pos + sz]))
        pos += sz
        use_a = not use_a
    return chunks


def _build_nc():
    import concourse.tile as tile
    from concourse import bacc, mybir

    bf16 = mybir.dt.bfloat16
    f16 = mybir.dt.float16
    f32 = mybir.dt.float32
    Exp = mybir.ActivationFunctionType.Exp

    nc = bacc.Bacc(
        "TRN2",
        target_bir_lowering=False,
        debug=False,
        enable_asserts=True,
        num_devices=N_CORES,
    )

    # partition-major host layouts so each load is ONE big DMA (DMA issue
    # on a queue costs ~650ns per instruction; many small DMAs starve the
    # prologue):
    #   xTr[p, jb*4096 + c*512 + col] = x.T[c*128+p, jb*512+col]
    #   wqkr[p, c*256 + u]            = wqk[c*128+p, u]
    #   wvr[p, c*128 + u]             = wv[c*128+p, u]
    xTr = nc.dram_tensor("xTr", [128, NCT * SEQ], bf16, kind="ExternalInput").ap()
    wqkr = nc.dram_tensor("wqkr", [128, NCT * 256], bf16, kind="ExternalInput").ap()
    wvr = nc.dram_tensor("wvr", [128, NCT * CBLK], bf16, kind="ExternalInput").ap()
    pw = nc.dram_tensor("pw", [CBLK, DMODEL], bf16, kind="ExternalInput").ap()
    bqk = nc.dram_tensor("bqk", [128, 2], f32, kind="ExternalInput").ap()
    partialT = nc.dram_tensor(
        "partialT", [DMODEL, SEQ], f16, kind="ExternalOutput"
    ).ap()
    if DEBUG_TAPS:
        dbg_kT = nc.dram_tensor("dbg_kT", [128, SEQ], bf16, kind="ExternalOutput").ap()
        dbg_e = nc.dram_tensor("dbg_e", [128, 4 * IT], bf16, kind="ExternalOutput").ap()
        dbg_av = nc.dram_tensor("dbg_av", [128, IT], f32, kind="ExternalOutput").ap()
        dbg_den = nc.dram_tensor("dbg_den", [16, IT], f32, kind="ExternalOutput").ap()
        dbg_rb = nc.dram_tensor("dbg_rb", [16, IT], f32, kind="ExternalOutput").ap()
        dbg_dacc = nc.dram_tensor("dbg_dacc", [16, IT], bf16, kind="ExternalOutput").ap()
        dbg_ao = nc.dram_tensor("dbg_ao", [128, SEQ], bf16, kind="ExternalOutput").ap()

    chunks = _chunk_pattern()

    with tile.TileContext(nc) as tc:
        with (
            tc.tile_pool(name="weights", bufs=1) as wpool,
            tc.tile_pool(name="xtiles", bufs=1) as xpool,
            tc.tile_pool(name="kq", bufs=1) as kqpool,
            tc.tile_pool(name="qtiles", bufs=3) as qpool,
            tc.tile_pool(name="vt", bufs=NJ) as vpool,
            tc.tile_pool(name="expA", bufs=10) as eapool,
            tc.tile_pool(name="expB", bufs=9) as ebpool,
            tc.tile_pool(name="dacc", bufs=8) as dpool,
            tc.tile_pool(name="attn", bufs=1) as apool,
            tc.tile_pool(name="norm", bufs=2) as npool,
            tc.tile_pool(name="stage", bufs=6) as stpool,
            tc.tile_pool(name="psA", bufs=1, space="PSUM") as psa,
            tc.tile_pool(name="psB", bufs=1, space="PSUM") as psb,
            tc.tile_pool(name="psV", bufs=1, space="PSUM") as psv,
            tc.tile_pool(name="psU", bufs=1, space="PSUM") as psu,
        ):
            # ---- ACT table warm-up: trigger the exp table load at t~0 ----
            warm_in = stpool.tile([1, 16], f32, name="warm_in")
            nc.vector.memset(warm_in[:], 0.0)
            warm_out = stpool.tile([1, 16], f32, name="warm_out")
            nc.scalar.activation(warm_out[:], warm_in[:], Exp)
            ones_t = wpool.tile([128, 1], bf16, name="ones_t")
            nc.vector.memset(ones_t[:], 1.0)

            # ---- weight + x DMAs: few big transfers, spread over 4 queues
            XB = NCT * JBLK  # 4096 columns of xt_all per j-block
            wqk_all = wpool.tile([128, NCT * 256], bf16, name="wqk_all")
            nc.sync.dma_start(wqk_all[:], wqkr[:])
            bqk_t = wpool.tile([128, 2], f32)
            nc.gpsimd.dma_start(bqk_t[:], bqk[:])  # tiny; gpsimd swdge is slow
            xt_all = xpool.tile([128, NCT * SEQ], bf16, name="xt_all", tag="xc")
            # jb0 in halves on the two fast queues: k(0)/q(0) start earlier
            nc.sync.dma_start(xt_all[:, 0 : XB // 2], xTr[:, 0 : XB // 2])
            nc.scalar.dma_start(xt_all[:, XB // 2 : XB], xTr[:, XB // 2 : XB])
            nc.scalar.dma_start(xt_all[:, XB : 2 * XB], xTr[:, XB : 2 * XB])
            wv_all = wpool.tile([128, NCT * CBLK], bf16, name="wv_all")
            nc.sync.dma_start(wv_all[:], wvr[:])
            pw_t = wpool.tile([128, DMODEL], bf16)
            nc.scalar.dma_start(pw_t[:], pw[:])
            dma_qs = [nc.sync, nc.scalar]
            for jb in range(2, NJB):
                dma_qs[jb % 2].dma_start(
                    xt_all[:, jb * XB : (jb + 1) * XB], xTr[:, jb * XB : (jb + 1) * XB]
                )

            def x_sl(c, jb, off, width):
                base = jb * XB + c * JBLK + off
                return xt_all[:, base : base + width]

            kT = kqpool.tile([128, SEQ], bf16)
            attn_outT = apool.tile([128, SEQ], bf16)
            qT = {}  # i -> tile
            vt = {}  # j -> tile [128, 128] = [v_h0 | v_h1]

            # rotating 1-bank psum tiles ("uv" doubles as the av/den bank)
            def util_tile(tag, name):
                pool = {"uv": psv, "u2": psu}[tag]
                return pool.tile([128, IT], f32, name=name, tag=tag)

            def _kq_mms(ps, woff, jb, lo, hi):
                for c in range(lo, hi):
                    nc.tensor.matmul(
                        ps[:],
                        wqk_all[:, c * 256 + woff : c * 256 + woff + 128],
                        x_sl(c, jb, 0, JBLK),
                        start=(c == 0),
                        stop=(c == NCT - 1),
                    )

            def emit_k(jb, tag):
                ps = util_tile(tag, f"kps{jb}")
                _kq_mms(ps, 128, jb, 0, NCT)
                nc.vector.tensor_scalar_add(
                    kT[:, jb * JBLK : (jb + 1) * JBLK], ps[:], bqk_t[:, 1:2]
                )

            def emit_q(i, tag):
                ps = util_tile(tag, f"qps{i}")
                _kq_mms(ps, 0, i, 0, NCT)
                qt = qpool.tile([128, IT], bf16, name=f"qT{i}", tag="qT")
                nc.vector.tensor_scalar_add(qt[:], ps[:], bqk_t[:, 0:1])
                qT[i] = qt

            def split_kq(kind, key, tag):
                """(first, second) side-work closures: 4 c-steps each, so a
                single pop never inserts a >1us matmul burst into the PE
                FIFO. First allocates the psum tile; second drains."""
                box = {}
                woff = 128 if kind == "k" else 0

                def first():
                    box["ps"] = util_tile(tag, f"{kind}ps{key}")
                    _kq_mms(box["ps"], woff, key, 0, 4)

                def second():
                    ps = box["ps"]
                    _kq_mms(ps, woff, key, 4, NCT)
                    if kind == "k":
                        nc.vector.tensor_scalar_add(
                            kT[:, key * JBLK : (key + 1) * JBLK], ps[:], bqk_t[:, 1:2]
                        )
                    else:
                        qt = qpool.tile([128, IT], bf16, name=f"qT{key}", tag="qT")
                        nc.vector.tensor_scalar_add(qt[:], ps[:], bqk_t[:, 0:1])
                        qT[key] = qt

                return first, second

            def emit_v(j, tag):
                ps = util_tile(tag, f"vps{j}")
                for c in range(NCT):
                    nc.tensor.matmul(
                        ps[:, 0:CBLK],
                        x_sl(c, j // 4, (j % 4) * JT, JT),
                        wv_all[:, c * CBLK : (c + 1) * CBLK],
                        start=(c == 0),
                        stop=(c == NCT - 1),
                    )
                va = vpool.tile([128, CBLK], bf16, name=f"vt{j}", tag="vt")
                nc.vector.tensor_copy(va[:], ps[:, 0:CBLK])
                vt[j] = va

            def emit_proj(cc, i, tag, use_act=False):
                ps = util_tile(tag, f"pp{cc}_{i}")
                nc.tensor.matmul(
                    ps[:],
                    pw_t[:, cc * 128 : (cc + 1) * 128],
                    attn_outT[:, i * IT : (i + 1) * IT],
                    start=True,
                    stop=True,
                )
                st = stpool.tile([128, IT], f16, name=f"st{cc}_{i}", tag="st")
                if use_act:  # tail only: ACT is idle once the exps are done
                    nc.scalar.copy(st[:], ps[:])
                else:
                    nc.vector.tensor_copy(st[:], ps[:])
                nc.sync.dma_start(
                    partialT[cc * 128 : (cc + 1) * 128, i * IT : (i + 1) * IT],
                    st[:],
                )

            # ---- prologue: k(0), q(0), v(0..3) only — everything else is
            # side work, so the PE FIFO reaches i0's score matmuls early.
            emit_k(0, "u2")
            emit_q(0, "uv")
            for j in range(4):
                emit_v(j, ["u2", "uv"][j % 2])

            # side work during the attention phase: runs on util bank u2
            # ONLY ("uv" is the av/den bank). Entries are (ready_slot,
            # closure): a closure is not popped before the global chunk-slot
            # counter reaches ready_slot, so work gated on slow chains
            # (normalize) never parks in the PE FIFO ahead of ready
            # attention matmuls.
            side_work = deque()
            slot = [0]
            # interleave k(1..7) with v(4..17): scores need k(jb) by slot
            # ~2.6*jb, av (elastically lagged) needs v(j) much later.
            for step in range(7):
                a, b = split_kq("k", step + 1, "u2")
                side_work.append((0, a))
                side_work.append((0, b))
                for j in (4 + 2 * step, 5 + 2 * step):
                    side_work.append((0, lambda j=j: emit_v(j, "u2")))
            a, b = split_kq("q", 1, "u2")
            side_work.append((0, a))
            side_work.append((0, b))
            for j in range(18, NJ):
                side_work.append((max(0, j - 14), lambda j=j: emit_v(j, "u2")))

            # ---- attention main loop ----
            av = {}  # i -> psum tile (h0 rows 0:64, h1 rows 64:128)
            den_acc = {}  # (i, h) -> sbuf bf16 running sum over j
            pending_av = deque()  # (i, e_tile, units)

            def emit_av_chunk(item):
                i, e_t, units = item
                for t, (j, h) in enumerate(units):
                    if i not in av:
                        av[i] = psv.tile([128, IT], f32, name=f"av_{i}", tag="uv")
                    # per-head accumulation groups: the has_written clear on
                    # start=True is per written partition-range, so each
                    # head must start its own group (measured: a shared
                    # group leaves h1 accumulating onto the previous
                    # i-tile's values).
                    nc.tensor.matmul(
                        av[i][h * 64 : (h + 1) * 64, :],
                        vt[j][:, h * 64 : h * 64 + 64],
                        e_t[:, t * IT : (t + 1) * IT],
                        start=(j == 0),
                        stop=(j == NJ - 1),
                        tile_position=(0, h * 64),
                    )
                    if j == NJ - 1 and h == 1:
                        finish_i(i)

            def finish_i(i):
                last = i == NI - 1
                # one copy drains both heads' av (frees the uv bank for
                # av(i+1)); the whole normalize chain is slot-delayed so no
                # engine FIFO ever head-blocks on the den accumulators
                # (which trail in the DVE/Pool queues).
                avs = npool.tile([128, IT], f32, name=f"avs_{i}", tag="avs")
                if last:
                    nc.scalar.copy(avs[:], av[i][:])
                else:
                    nc.vector.tensor_copy(avs[:], av[i][:])
                del av[i]
                if DEBUG_TAPS and i == 0:
                    nc.sync.dma_start(dbg_av[:], avs[:])
                rb = npool.tile([128, IT], f32, name=f"rb_{i}", tag="rb")

                def den_chain(i=i, rb=rb):
                    denp = psu.tile([128, IT], f32, name=f"den_{i}", tag="u2")
                    for h in range(2):
                        nc.tensor.matmul(
                            denp[h * 32 : h * 32 + 1, :],
                            ones_t[:, 0:1],
                            den_acc[(i, h, "v")][:],
                            start=True,
                            stop=True,
                            tile_position=(0, h * 32),
                        )
                    for h in range(2):
                        # den to partition 0 SBUF: reciprocal_approx_fast
                        # (custom DVE op) misreads partition-offset inputs.
                        dent = npool.tile([1, IT], f32, name=f"den{h}_{i}", tag="dent")
                        if last and h == 1:
                            nc.scalar.copy(dent[:], denp[h * 32 : h * 32 + 1, :])
                        else:
                            nc.vector.tensor_copy(dent[:], denp[h * 32 : h * 32 + 1, :])
                        rd = npool.tile([1, IT], f32, name=f"rd{h}_{i}", tag="rd")
                        nc.vector.reciprocal_approx_fast(rd[:], dent[:])
                        if h == 0:
                            nc.gpsimd.partition_broadcast(
                                rb[0:64, :], rd[:, 0:IT], channels=64
                            )
                        else:
                            # partition_broadcast mis-writes base-64 outputs;
                            # broadcast at base 0 and copy up.
                            rbt = npool.tile([64, IT], f32, name=f"rbt_{i}", tag="rbt")
                            nc.gpsimd.partition_broadcast(
                                rbt[:], rd[:, 0:IT], channels=64
                            )
                            nc.vector.tensor_copy(rb[64:128, :], rbt[:])
                        if DEBUG_TAPS:
                            nc.sync.dma_start(
                                dbg_den[2 * i + h : 2 * i + h + 1, :], dent[:]
                            )

                def mul_norm(i=i, avs=avs, rb=rb):
                    nc.vector.tensor_mul(
                        attn_outT[:, i * IT : (i + 1) * IT], avs[:], rb[:]
                    )

                if last:
                    den_chain()
                    mul_norm()
                else:
                    side_work.append((slot[0] + 3, den_chain))
                    if i + 2 < NI:
                        a, b = split_kq("q", i + 2, "u2")
                        side_work.append((slot[0] + 4, a))
                        side_work.append((slot[0] + 5, b))
                    side_work.append((slot[0] + 6, mul_norm))
                rdy = slot[0] + 8
                if last:
                    # tail: uv bank is free after the den matmuls — rotate
                    # proj across both util banks and alternate the drain
                    # between DVE and the now-idle ACT engine.
                    rot = ["u2", "uv"]
                    for cc in range(NCT):
                        side_work.append(
                            (
                                slot[0],
                                lambda cc=cc, i=i: emit_proj(
                                    cc, i, rot[cc % 2], use_act=(cc % 2 == 1)
                                ),
                            )
                        )
                else:
                    # stagger: >=2 chunk-slots between proj matmuls so they
                    # never crowd the PE FIFO ahead of the next scores
                    for cc in range(NCT):
                        side_work.append(
                            (rdy + 2 * cc, lambda cc=cc, i=i: emit_proj(cc, i, "u2"))

                        )

            for i in range(NI):
                # elastic av lag: defer av matmuls early so i0/i1 PE slack
                # absorbs the one-time k/v/q side work without stalling ACT
                lag = {0: 8, 1: 6, 2: 4}.get(i, 3)
                for ck, (kind, units) in enumerate(chunks):
                    csz = len(units)
                    if kind == "A":
                        sc = psa.tile(
                            [128, CSZ_A * IT], f32, name=f"scA_{i}_{ck}", tag="scA"
                        )
                    else:
                        sc = psb.tile(
                            [128, CSZ_B * IT], f32, name=f"scB_{i}_{ck}", tag="scB"
                        )
                    for t, (j, h) in enumerate(units):
                        nc.tensor.matmul(
                            sc[:, t * IT : (t + 1) * IT],
                            kT[h * 64 : (h + 1) * 64, j * JT : (j + 1) * JT],
                            qT[i][h * 64 : (h + 1) * 64, :],
                            start=True,
                            stop=True,
                            tile_position=(h * 64, 0),
                        )
                    epool = eapool if kind == "A" else ebpool
                    e_t = epool.tile(
                        [128, csz * IT], bf16, name=f"e_{i}_{ck}", tag=f"e{kind}"
                    )
                    nc.scalar.activation(
                        e_t[:, 0 : csz * IT], sc[:, 0 : csz * IT], Exp, scale=SCALE
                    )
                    if DEBUG_TAPS and i == 0 and ck == 0:
                        nc.sync.dma_start(dbg_e[:], e_t[:])
                    # denominator running sums, split DVE (j%3!=2) and the
                    # otherwise-idle gpsimd engine (j%3==2) into two partial
                    # accumulators; the fp32 ones-matmul pair sums them and
                    # reduces across partitions exactly.
                    for t, (j, h) in enumerate(units):
                        sl = e_t[:, t * IT : (t + 1) * IT]
                        eng, key = (nc.vector, "v")  # Pool offload measured slower
                        if (i, h, key) not in den_acc:
                            da = dpool.tile(
                                [128, IT], bf16, name=f"dacc{key}{h}_{i}", tag="dacc"
                            )
                            eng.tensor_copy(da[:], sl)
                            den_acc[(i, h, key)] = da
                        else:
                            da = den_acc[(i, h, key)]
                            eng.tensor_add(da[:], da[:], sl)
                    pending_av.append((i, e_t, units))
                    navpop = 0
                    while len(pending_av) > lag and navpop < 2:
                        emit_av_chunk(pending_av.popleft())
                        navpop += 1
                    slot[0] += 1
                    npop = 2 if i == 0 else 1
                    for _ in range(npop):
                        if side_work and side_work[0][0] <= slot[0]:
                            side_work.popleft()[1]()

            while pending_av:
                emit_av_chunk(pending_av.popleft())
                slot[0] += 1
                if side_work and side_work[0][0] <= slot[0]:
                    side_work.popleft()[1]()
            while side_work:
                side_work.popleft()[1]()
            if DEBUG_TAPS:
                nc.sync.dma_start(dbg_kT[:], kT[:])
                nc.sync.dma_start(dbg_ao[:], attn_outT[:])

    nc.compile()
    return nc


def _get_nc():
    if "nc" not in _CACHE:
        _CACHE["nc"] = _build_nc()
    return _CACHE["nc"]


def build_in_maps(x, qkv_w, qkv_b, proj_w):
    """Per-core input dicts in the device's partition-major layouts:
    xTr[p, jb*4096+c*512+col] = x.T[c*128+p, jb*512+col], and the c-chunks
    of wqk/wv concatenated along columns with partitions inside each chunk.
    """
    bf16 = ml_dtypes.bfloat16
    x2d = np.ascontiguousarray(np.asarray(x).reshape(SEQ, DMODEL).T)  # [1024, 4096]
    xTr = np.ascontiguousarray(
        x2d.reshape(NCT, 128, NJB, JBLK).transpose(1, 2, 0, 3).reshape(128, NCT * SEQ)
    ).astype(bf16)

    def cmajor(w):  # [1024, u] -> [128, 8*u]
        u = w.shape[1]
        return np.ascontiguousarray(
            np.asarray(w).reshape(NCT, 128, u).transpose(1, 0, 2).reshape(128, NCT * u)
        ).astype(bf16)

    in_maps = []
    for c in range(N_CORES):
        lo, hi = c * CBLK, (c + 1) * CBLK
        wq_c = np.asarray(qkv_w[lo:hi, :])  # [128, 1024]
        wk_c = np.asarray(qkv_w[DMODEL + lo : DMODEL + hi, :])
        wv_c = np.asarray(qkv_w[2 * DMODEL + lo : 2 * DMODEL + hi, :])
        in_maps.append(
            {
                "xTr": xTr,
                "wqkr": cmajor(np.concatenate([wq_c.T, wk_c.T], axis=1)),
                "wvr": cmajor(wv_c.T),
                "pw": np.ascontiguousarray(np.asarray(proj_w)[:, lo:hi].T).astype(
                    bf16
                ),
                "bqk": np.ascontiguousarray(
                    np.stack(
                        [qkv_b[lo:hi], qkv_b[DMODEL + lo : DMODEL + hi]], axis=1
                    )
                ).astype(np.float32),
            }
        )
    return in_maps


def kernel(x, qkv_w, qkv_b, proj_w, proj_b):
    from concourse.bass_utils import run_bass_kernel_spmd

    nc = _get_nc()
    in_maps = build_in_maps(x, qkv_w, qkv_b, proj_w)
    res = run_bass_kernel_spmd(nc, in_maps, core_ids=list(range(N_CORES)))

    acc = np.zeros((DMODEL, SEQ), dtype=np.float32)
    for c in range(N_CORES):
        acc += res.results[c]["partialT"].astype(np.float32)

    # host-side linear bias terms: proj bias + v-bias routed through proj
    bias = qkv_b[2 * DMODEL :].astype(np.float32) @ proj_w.T.astype(
        np.float32
    ) + proj_b.astype(np.float32)
    out = acc.T + bias[None, :]
    return out.reshape(1, SEQ, DMODEL).astype(np.float32)
